# revision 20
# baseline (speedup 1.0000x reference)
"""Trainium2 Bass kernel for masked pairwise-sigmoid GNN message passing.

Reference computation (per graph g with nodes i,j in [0,nv)):
    c = z @ Wc.T + bc ; y = z @ Wy.T + by          # [G, nv, H]
    s[g,i,j,:] = sigmoid(c[g,i,:] + y[g,j,:] + (m_i + m_j)*L - 2L)
    out[g,i,:] = sum_j s[g,i,j,:] / sum_j m[g,j]

Exact identity: with m in {0,1}, any pair with m_i==0 or m_j==0 has mask
term <= -1e10, so sigmoid underflows to exactly 0 in fp32.  Only active
nodes (m==1) contribute; for active pairs the mask term is 0.  The host
gathers active nodes per graph, the device computes the dense active x
active interaction, and the host scatters rows back (applying the
1/n_active scale during the scatter).

Sharding: graphs sorted by active count, dealt round-robin to 8 cores in
4 slots; slot s padded to a global j-extent P_s (multiple of 4) and an
exact i-extent PI_s, so one SPMD program serves all cores.  Padding
columns get a -1e5 additive mask (sigmoid -> 0); padding rows are
discarded on scatter.

Device design (v3; every choice below is from perfetto traces on HW):
- Each dma_start costs ~640ns of issue plus ~1.3us of queue pipeline
  before data moves, so DMAs are consolidated: sync carries zT then wcT;
  gpsimd carries wyT then a 1-row aux [em | ones | bc+by].  The scalar
  queue carries NO input DMA: a dma_start on the scalar engine ahead of
  the first activation makes the act-table pass emit a second
  ACT_TABLE_LOAD (sets are chosen greedily per function), and with a
  dummy sigmoid leading the scalar stream the single sigmoid_and_others
  load (contains copy too) runs during the DMA window instead.
- bc+by and the pad mask are folded into the y-projection PSUM as two
  rank-1 matmuls (ones (x) em  and  bsum (x) ones), so all PSUM
  evacuations are plain table-free Copy ops and sigmoids need no bias.
- c' is evacuated by the DVE (idle until pass 1) as duplicated bf16
  pairs [c_i|c_i] straight from PSUM — the step-1 pairs are what enables
  the 2x_1P packed mode for the broadcast add; y' is evacuated by ACT.
- Reduction per slot: sigmoid (ACT, (224+FD)/1.2GHz, the only engine
  with a table path) then bf16 halving adds on DVE at 2 elem/cycle —
  twice when P%8==0 (the second halve's operand offset must stay
  4B-aligned) — and a final tensor_reduce (1 elem/cycle, never packs).
- GPSIMD issues DMAs only (a running gpsimd tensor op blocks
  concurrently issued 2-port DVE ops on the shared POOL SBUF port).
"""

import numpy as np

import concourse.bass as bass
import concourse.mybir as mybir
import concourse.tile as tile
from concourse import bacc
from concourse.bass_utils import run_bass_kernel_spmd

F32 = mybir.dt.float32
BF16 = mybir.dt.bfloat16
N_CORES = 8
PAD_NEG = -1.0e5  # additive mask for padding columns; sigmoid(-1e5) == 0

# test.py reads this for profiling info after a traced run
_last_results = None
_program_cache = {}


def _ap(view, free_dims):
    """AP anchored at `view`'s base with custom free dims (stride, num)."""
    return bass.AP(
        tensor=view.tensor,
        offset=view.offset,
        ap=[list(view.ap[0])] + [[int(s), int(n)] for s, n in free_dims],
    )


def _build_program(P_list, PI_list, H):
    """P_list: per-slot j-extent (mult of 4); PI_list: per-slot i-extent."""
    NTOT = sum(P_list)
    ONB = max(128, NTOT)  # ones block length (lhsT slice needs >=128)
    assert H == 256
    nc = bacc.Bacc(None, target_bir_lowering=False)

    # sync blob = [z (2*NTOT) | wc chunks (512) | wy chunks (512)]; weight
    # chunk layout per W: (kb0ob0 | kb1ob0 | kb0ob1 | kb1ob1) * 128
    BW = 2 * NTOT + 4 * H
    blob = nc.dram_tensor("blob", [128, BW], BF16, kind="ExternalInput")
    # two rows, the k=2 rank-1 operands (see below):
    #   row0: [em (NTOT)   | ones (128) | ones (128)]
    #   row1: [ones (NTOT) | bsum ob0   | bsum ob1  ]
    aux = nc.dram_tensor("aux", [2, NTOT + 256], BF16, kind="ExternalInput")
    out = nc.dram_tensor("out", [128, 2 * NTOT], F32, kind="ExternalOutput")

    AT = mybir.ActivationFunctionType
    OP = mybir.AluOpType

    offs = [0]
    for P in P_list[:-1]:
        offs.append(offs[-1] + P)

    with tile.TileContext(nc) as tc:
        with (
            tc.tile_pool(name="singles", bufs=1) as singles,
            tc.tile_pool(name="ptp", bufs=2) as ptp,
            tc.tile_pool(name="stp", bufs=2) as stp,
            tc.tile_pool(name="hvp", bufs=2) as hvp,
            tc.tile_pool(name="hqp", bufs=2) as hqp,
            tc.tile_pool(name="oup", bufs=2) as oup,
            tc.tile_pool(name="psum", bufs=1, space="PSUM") as psum,
        ):
            blob_sb = singles.tile([128, BW], BF16, tag="blob", name="blob_sb")
            z_sb = blob_sb[:, 0:2 * NTOT]
            w_sb = {
                "c": blob_sb[:, 2 * NTOT:2 * NTOT + 2 * H],
                "y": blob_sb[:, 2 * NTOT + 2 * H:2 * NTOT + 4 * H],
            }
            aux_sb = singles.tile([2, NTOT + 256], BF16, tag="aux", name="aux_sb")
            scratch = singles.tile([1, 1], F32, tag="scr", name="scratch")

            # sync: everything big in ONE dma (a queue's 2nd DMA starts its
            # data ~1.1us after the 1st ends); gpsimd: the tiny aux rows
            # (land ~1us before the blob, so the rank-1s run first);
            # scalar: NO dma (it would force a 2nd ACT_TABLE_LOAD)
            nc.sync.dma_start(out=blob_sb[:], in_=blob[:])
            nc.gpsimd.dma_start(out=aux_sb[:], in_=aux[:])
            # dummy sigmoid: pins the single table load at the top of the
            # scalar stream, overlapping the DMA window
            nc.scalar.activation(
                out=scratch[:], in_=nc.const_aps.tensor(0.0, (1, 1)),
                func=AT.Sigmoid,
            )

            # ---- projections -> PSUM (biases+mask folded in as ONE k=2
            # matmul: rows [ones;bsum] x [em;ones] add em[col] + bsum[h])
            ps_t = {}
            for ob in range(2):
                for wname in ("c", "y"):
                    ps = psum.tile(
                        [128, NTOT], F32, tag=f"ps{wname}{ob}", name=f"ps{wname}{ob}"
                    )
                    if wname == "y":
                        # rank-1 terms first: aux lands ~1us before the blob,
                        # so the PE retires this before the weights arrive
                        nc.tensor.matmul(
                            ps[:],
                            lhsT=aux_sb[0:2, NTOT + 128 * ob:NTOT + 128 * (ob + 1)],
                            rhs=aux_sb[0:2, 0:NTOT],
                            start=True, stop=False,
                        )
                    for kb in range(2):
                        o0 = (2 * ob + kb) * 128
                        nc.tensor.matmul(
                            ps[:],
                            lhsT=w_sb[wname][:, o0:o0 + 128],
                            rhs=z_sb[:, kb * NTOT:(kb + 1) * NTOT],
                            start=(kb == 0 and wname == "c"),
                            stop=(kb == 1),
                        )
                    ps_t[wname, ob] = ps

            # ---- evacuations: c' by DVE (idle) as dup pairs; y' by ACT
            c2 = singles.tile([128, 4 * NTOT], BF16, tag="c2", name="c2")
            yb = singles.tile([128, 2 * NTOT], BF16, tag="yb", name="yb")
            for ob in range(2):
                nc.vector.tensor_copy(
                    out=_ap(c2[:, 2 * ob * NTOT:2 * ob * NTOT + 2],
                            [(2, NTOT), (1, 2)]),
                    in_=_ap(ps_t["c", ob][:, 0:NTOT], [(1, NTOT), (0, 2)]),
                )
                nc.scalar.copy(
                    out=yb[:, ob * NTOT:(ob + 1) * NTOT],
                    in_=ps_t["y", ob][:, 0:NTOT],
                )

            # ---- pass 1: packed broadcast adds, SMALLEST slot first (the
            # sigmoid chain hangs off the first TT pair; ascending order
            # starts it ~1us earlier and parks the biggest tree at the end
            # where the DVE has slack)
            # smallest slot first (sigmoid chain starts earliest), then the
            # big slots, second-smallest last (its tree caps the stream)
            asc = sorted(range(len(P_list)), key=lambda s: PI_list[s] * P_list[s])
            sorder = [asc[0]] + asc[:1:-1] + asc[1:2]
            pts = {}
            for si in sorder:
                P, PI = P_list[si], PI_list[si]
                col = offs[si]
                pt = ptp.tile(
                    [128, 2, PI, P], BF16, tag=f"pair{si}", name=f"pair{si}"
                )
                for ob in range(2):
                    cb = 2 * ob * NTOT + 2 * col
                    in0 = _ap(c2[:, cb:cb + 2], [(2, PI), (0, P // 2), (1, 2)])
                    in1 = _ap(yb[:, ob * NTOT + col:ob * NTOT + col + P],
                              [(0, PI), (1, P)])
                    nc.vector.tensor_tensor(
                        out=pt[:, ob:ob + 1], in0=in0, in1=in1, op=OP.add
                    )
                pts[si] = pt

            # ---- pass 2: sigmoid -> halving tree -> reduce -> store
            for si in sorder:
                P, PI = P_list[si], PI_list[si]
                col = offs[si]
                pt = pts[si]
                st = stp.tile([128, 2, PI, P], BF16, tag="sig", name="sig_t")
                nc.scalar.activation(out=st[:], in_=pt[:], func=AT.Sigmoid)
                hw = P // 2
                hv = hvp.tile([128, 2, PI, hw], BF16, tag="hv", name="hv_t")
                nc.vector.tensor_tensor(
                    out=hv[:], in0=st[:, :, :, 0:hw], in1=st[:, :, :, hw:P],
                    op=OP.add,
                )
                last = hv
                w = hw
                if P % 8 == 0:  # second halve stays 4B-aligned only then
                    hq = hw // 2
                    h2 = hqp.tile([128, 2, PI, hq], BF16, tag="hq", name="hq_t")
                    nc.vector.tensor_tensor(
                        out=h2[:], in0=hv[:, :, :, 0:hq], in1=hv[:, :, :, hq:hw],
                        op=OP.add,
                    )
                    last, w = h2, hq
                red = oup.tile([128, 2, PI], F32, tag="red", name="red_t")
                nc.vector.reduce_sum(out=red[:], in_=last[:], axis=mybir.AxisListType.X)
                nc.sync.dma_start(
                    out=_ap(out[0:128, col:col + PI], [(NTOT, 2), (1, PI)]),
                    in_=red[:],
                )

    nc.finalize()
    return nc


def kernel(num_graphs, nv, z, mask, Wc, bc, Wy, by):
    global _last_results
    G = int(num_graphs)
    NV = int(nv)
    z = np.ascontiguousarray(np.asarray(z, dtype=np.float32))
    mask = np.asarray(mask, dtype=np.float32).reshape(G, NV)
    Wc = np.asarray(Wc, dtype=np.float32)
    bc = np.asarray(bc, dtype=np.float32)
    Wy = np.asarray(Wy, dtype=np.float32)
    by = np.asarray(by, dtype=np.float32)
    H = z.shape[-1]
    zg = z.reshape(G, NV, H)

    out_full = np.zeros((G * NV, H), dtype=np.float32)

    # ---- host: active-node compaction & slot assignment ----
    act_idx = [np.nonzero(mask[g] > 0.5)[0] for g in range(G)]
    n_act = np.array([len(a) for a in act_idx])
    for g in range(G):
        if n_act[g] == 0:  # reference: 0/0 -> NaN for the whole graph
            out_full[g * NV:(g + 1) * NV, :] = np.nan

    order = np.argsort(-n_act, kind="stable")  # graphs by count, descending
    n_slots = (G + N_CORES - 1) // N_CORES
    assign = [[None] * n_slots for _ in range(N_CORES)]
    P_list = []
    for s in range(n_slots):
        ranks = order[s * N_CORES:(s + 1) * N_CORES]
        for c, g in enumerate(ranks):
            assign[c][s] = int(g)
        mx = max((int(n_act[g]) for g in ranks), default=0)
        P_list.append(max(4, (mx + 3) // 4 * 4))  # j-extent: multiple of 4
    PI_list = [max(1, max((int(n_act[g]) for g in order[s * N_CORES:(s + 1) * N_CORES]), default=1)) for s in range(n_slots)]
    offs = np.cumsum([0] + P_list[:-1]).tolist()
    NTOT = sum(P_list)
    ONB = max(128, NTOT)

    # ---- host: per-core input staging ----
    import ml_dtypes

    def _wchunks(wt):  # [256, 256] -> [128, 512] chunks (kb,ob)-major for ob0 first
        w2 = np.empty((128, 512), dtype=ml_dtypes.bfloat16)
        for ob in range(2):
            for kb in range(2):
                w2[:, (2 * ob + kb) * 128:(2 * ob + kb + 1) * 128] = (
                    wt[kb * 128:(kb + 1) * 128, ob * 128:(ob + 1) * 128]
                )
        return np.ascontiguousarray(w2)

    wcT = _wchunks(Wc.T.astype(ml_dtypes.bfloat16))  # [h_in, o] chunks
    wyT = _wchunks(Wy.T.astype(ml_dtypes.bfloat16))
    bsum = (bc + by).astype(np.float32)

    in_maps = []
    for c in range(N_CORES):
        zT_act = np.zeros((H, NTOT), dtype=ml_dtypes.bfloat16)
        madd = np.full((1, NTOT), PAD_NEG, dtype=np.float32)
        for s in range(n_slots):
            g = assign[c][s]
            if g is None:
                continue
            n = int(n_act[g])
            if n == 0:
                continue
            o = int(offs[s])
            zT_act[:, o:o + n] = zg[g][act_idx[g]].T.astype(ml_dtypes.bfloat16)
            madd[0, o:o + n] = 0.0
        blob = np.empty((128, 2 * NTOT + 1024), dtype=ml_dtypes.bfloat16)
        blob[:, :NTOT] = zT_act[:128]
        blob[:, NTOT:2 * NTOT] = zT_act[128:]
        blob[:, 2 * NTOT:2 * NTOT + 512] = wcT
        blob[:, 2 * NTOT + 512:2 * NTOT + 1024] = wyT
        auxrow = np.zeros((2, NTOT + 256), dtype=ml_dtypes.bfloat16)
        auxrow[0, 0:NTOT] = madd[0].astype(ml_dtypes.bfloat16)
        auxrow[0, NTOT:NTOT + 256] = 1.0
        auxrow[1, 0:NTOT] = 1.0
        auxrow[1, NTOT:NTOT + 256] = bsum.astype(ml_dtypes.bfloat16)
        in_maps.append(
            {
                "blob": np.ascontiguousarray(blob),
                "aux": np.ascontiguousarray(auxrow),
            }
        )

    # ---- build + run ----
    key = (tuple(P_list), tuple(PI_list), H)
    nc = _program_cache.get(key)
    if nc is None:
        nc = _build_program(P_list, PI_list, H)
        _program_cache[key] = nc
    res = run_bass_kernel_spmd(nc, in_maps, list(range(N_CORES)))
    _last_results = res

    # ---- host: scatter back (device output is [h1, (ob, col)]-major) ----
    for c in range(N_CORES):
        oc = res.results[c]["out"].reshape(128, 2, NTOT)  # [h1, ob, col]
        for s in range(n_slots):
            g = assign[c][s]
            if g is None:
                continue
            n = int(n_act[g])
            if n == 0:
                continue
            o = int(offs[s])
            blk = oc[:, :, o:o + n]  # [128, 2, n] (unscaled sums)
            out_full[g * NV + act_idx[g], :] = (
                blk.transpose(2, 1, 0).reshape(n, H)
                * (np.float32(1.0) / np.float32(n))
            )
    return out_full


# revision 41
# speedup vs baseline: 1.0099x; 1.0099x over previous
"""Trainium2 Bass kernel for masked pairwise-sigmoid GNN message passing.

Reference computation (per graph g with nodes i,j in [0,nv)):
    c = z @ Wc.T + bc ; y = z @ Wy.T + by          # [G, nv, H]
    s[g,i,j,:] = sigmoid(c[g,i,:] + y[g,j,:] + (m_i + m_j)*L - 2L)
    out[g,i,:] = sum_j s[g,i,j,:] / sum_j m[g,j]

Exact identity: with m in {0,1}, any pair with m_i==0 or m_j==0 has mask
term <= -1e10, so sigmoid underflows to exactly 0 in fp32.  Only active
nodes (m==1) contribute; for active pairs the mask term is 0.  The host
gathers active nodes per graph, the device computes the dense active x
active interaction, and the host scatters rows back (applying the
1/n_active scale during the scatter).

Sharding: graphs sorted by active count, dealt round-robin to 8 cores in
4 slots; slot s padded to a global j-extent P_s (multiple of 4) and an
exact i-extent PI_s, so one SPMD program serves all cores.  Padding
columns get a -1e5 additive mask (sigmoid -> 0); padding rows are
discarded on scatter.

Device design (v4; every choice below is from perfetto traces on HW):
- A dma_start costs ~640ns of engine issue plus ~1.2us of queue pipe
  before data moves, and a queue's 2nd DMA starts its data ~1.1us after
  the 1st ends.  So each queue carries ONE real input: sync gets the
  blob [z | wc chunks], the scalar queue gets wy, gpsimd gets the tiny
  2-row aux.  The scalar DMA sits ahead of the dummy sigmoid in the
  stream, which makes the act-table pass emit set-0 + set-2 loads — but
  both run inside the DMA window and nothing needs ACT before the first
  real sigmoid, so they are free; the dma issue itself overlaps the
  table load on the engine.
- bc+by and the -1e5 pad-column mask are folded into the y-projection
  PSUM as ONE k=2 matmul ([ones;bsum] x [em;ones]), so the evacuations
  are bias-free and sigmoids need no bias AP.
- c'/y' are evacuated into (col, ob)-interleaved bf16 layouts
  [v_ob0|v_ob1] — c' by the DVE (idle before pass 1), y' by ACT.  The
  ob-pair innermost gives pass 1 ONE tensor_tensor per slot (both
  output-halves at once) with the step-1 innermost pair the 2x_1P mode
  needs, and makes every halving-tree offset 4B-aligned (offset 2j
  elems = 4j bytes), so h2 packs at 2x for every P.  The sigmoid and
  tree stay contiguous in pairs layout (a strided ACT output measured
  ~4x slower); only the stride-insensitive 1x tensor_reduce iterates
  (i, ob, j) via its AP to reduce j and emit ob-major f32 for the
  store.
- Slot order [smallest, largest, ..., 2nd-smallest]: the sigmoid chain
  hangs off the first slot's TT pair (smallest first starts it
  earliest) and the last slot pays its tree serially after the final
  sigmoid (2nd-smallest caps that tail).
- Reduction per slot: sigmoid (ACT, (224+FD)/1.2GHz, the only engine
  with a table path) then two bf16 halving adds on DVE at 2 elem/cycle
  and the final tensor_reduce (1 elem/cycle, never packs); the last
  slot reduces+stores per ob so the tail-bounding store is half-sized.
- Pass-2 pools use bufs=4: with bufs=2 the tensor_reduces stall ~1us
  each waiting for earlier slots' store DMA round-trips to free the
  red/hv buffers.
- GPSIMD issues DMAs only: a running gpsimd tensor op blocks
  concurrently issued 2-port DVE ops (shared POOL SBUF port), and its
  tensor_reduce cannot reduce the free axis anyway.
- The DVE stream (c-casts, 8 pass-1 adds, per-slot trees) is ~100% busy
  from first cast to last reduce; it, the ~6.9us NEFF preamble, and the
  ~4us store/teardown tail bound the kernel.
"""

import numpy as np

import concourse.bass as bass
import concourse.mybir as mybir
import concourse.tile as tile
from concourse import bacc
from concourse.bass_utils import run_bass_kernel_spmd

F32 = mybir.dt.float32
BF16 = mybir.dt.bfloat16
N_CORES = 8
PAD_NEG = -1.0e5  # additive mask for padding columns; sigmoid(-1e5) == 0

# test.py reads this for profiling info after a traced run
_last_results = None
_program_cache = {}


def _ap(view, free_dims):
    """AP anchored at `view`'s base with custom free dims (stride, num)."""
    return bass.AP(
        tensor=view.tensor,
        offset=view.offset,
        ap=[list(view.ap[0])] + [[int(s), int(n)] for s, n in free_dims],
    )


def _build_program(P_list, PI_list, H):
    """P_list: per-slot j-extent (mult of 4); PI_list: per-slot i-extent."""
    NTOT = sum(P_list)
    ONB = max(128, NTOT)  # ones block length (lhsT slice needs >=128)
    assert H == 256
    nc = bacc.Bacc(None, target_bir_lowering=False)

    # sync blob = [z (2*NTOT) | wc chunks (512)]; wy rides the scalar
    # queue (its engine-stream position before the dummy sigmoid makes the
    # act-table pass emit set-0+set-2 loads, but both run inside the DMA
    # window and nothing on ACT is needed before the first real sigmoid).
    # Weight chunk layout per W: (kb0ob0 | kb1ob0 | kb0ob1 | kb1ob1) * 128
    BW = 2 * NTOT + 2 * H
    blob = nc.dram_tensor("blob", [128, BW], BF16, kind="ExternalInput")
    wyT = nc.dram_tensor("wyT", [128, 2 * H], BF16, kind="ExternalInput")
    # two rows, the k=2 rank-1 operands (see below):
    #   row0: [em (NTOT)   | ones (128) | ones (128)]
    #   row1: [ones (NTOT) | bsum ob0   | bsum ob1  ]
    aux = nc.dram_tensor("aux", [2, NTOT + 256], BF16, kind="ExternalInput")
    out = nc.dram_tensor("out", [128, 2 * NTOT], F32, kind="ExternalOutput")

    AT = mybir.ActivationFunctionType
    OP = mybir.AluOpType

    offs = [0]
    for P in P_list[:-1]:
        offs.append(offs[-1] + P)

    with tile.TileContext(nc) as tc:
        with (
            tc.tile_pool(name="singles", bufs=1) as singles,
            tc.tile_pool(name="ptp", bufs=4) as ptp,
            tc.tile_pool(name="stp", bufs=4) as stp,
            tc.tile_pool(name="hvp", bufs=4) as hvp,
            tc.tile_pool(name="hqp", bufs=4) as hqp,
            tc.tile_pool(name="oup", bufs=4) as oup,
            tc.tile_pool(name="psum", bufs=1, space="PSUM") as psum,
        ):
            blob_sb = singles.tile([128, BW], BF16, tag="blob", name="blob_sb")
            z_sb = blob_sb[:, 0:2 * NTOT]
            wy_sb = singles.tile([128, 2 * H], BF16, tag="wy", name="wy_sb")
            w_sb = {
                "c": blob_sb[:, 2 * NTOT:2 * NTOT + 2 * H],
                "y": wy_sb,
            }
            aux_sb = singles.tile([2, NTOT + 256], BF16, tag="aux", name="aux_sb")
            scratch = singles.tile([1, 1], F32, tag="scr", name="scratch")

            # sync: everything big in ONE dma (a queue's 2nd DMA starts its
            # data ~1.1us after the 1st ends); gpsimd: the tiny aux rows
            # (land ~1us before the blob, so the rank-1s run first);
            # scalar: NO dma (it would force a 2nd ACT_TABLE_LOAD)
            nc.sync.dma_start(out=blob_sb[:], in_=blob[:])
            nc.scalar.dma_start(out=wy_sb[:], in_=wyT[:])
            nc.gpsimd.dma_start(out=aux_sb[:], in_=aux[:])
            # dummy sigmoid: pins the single table load at the top of the
            # scalar stream, overlapping the DMA window
            nc.scalar.activation(
                out=scratch[:], in_=nc.const_aps.tensor(0.0, (1, 1)),
                func=AT.Sigmoid,
            )

            # ---- projections -> PSUM (biases+mask folded in as ONE k=2
            # matmul: rows [ones;bsum] x [em;ones] add em[col] + bsum[h])
            ps_t = {}
            for ob in range(2):
                for wname in ("c", "y"):
                    ps = psum.tile(
                        [128, NTOT], F32, tag=f"ps{wname}{ob}", name=f"ps{wname}{ob}"
                    )
                    for kb in range(2):
                        o0 = (2 * ob + kb) * 128
                        nc.tensor.matmul(
                            ps[:],
                            lhsT=w_sb[wname][:, o0:o0 + 128],
                            rhs=z_sb[:, kb * NTOT:(kb + 1) * NTOT],
                            start=(kb == 0),
                            stop=(kb == 1 and wname == "c"),
                        )
                    if wname == "y":
                        nc.tensor.matmul(
                            ps[:],
                            lhsT=aux_sb[0:2, NTOT + 128 * ob:NTOT + 128 * (ob + 1)],
                            rhs=aux_sb[0:2, 0:NTOT],
                            start=False, stop=True,
                        )
                    ps_t[wname, ob] = ps

            # ---- evacuations into (col, ob)-interleaved layouts: c2/yb hold
            # [v_ob0(col)|v_ob1(col)] pairs, so ONE pass-1 TT per slot covers
            # both output-halves with the step-1 innermost pair the 2x_1P
            # mode needs (and any j offset stays 4B-aligned down the tree).
            # c' by DVE (idle until pass 1); y' by ACT, first slot's block
            # first so the first TT starts as early as possible.
            c2 = singles.tile([128, 2 * NTOT], BF16, tag="c2", name="c2")
            yb = singles.tile([128, 2 * NTOT], BF16, tag="yb", name="yb")
            asc = sorted(range(len(P_list)), key=lambda s: PI_list[s] * P_list[s])
            sorder = [asc[0]] + asc[:1:-1] + asc[1:2]
            sf = sorder[0]
            o_sf, P_sf = offs[sf], P_list[sf]
            assert o_sf + P_sf == NTOT  # smallest slot is the last block
            def _pass1(si):
                P, PI = P_list[si], PI_list[si]
                col = offs[si]
                pt = ptp.tile(
                    [128, PI, P, 2], BF16, tag=f"pair{si}", name=f"pair{si}"
                )
                in0 = _ap(c2[:, 2 * col:2 * col + 2], [(2, PI), (0, P), (1, 2)])
                in1 = _ap(yb[:, 2 * col:2 * col + 2], [(0, PI), (2, P), (1, 2)])
                nc.vector.tensor_tensor(
                    out=pt[:], in0=in0, in1=in1, op=OP.add
                )
                return pt

            # ---- pass 1 interleaved with the evacuations: the first
            # (smallest) slot's block evacuates first, its TT issues, and
            # only then the rest-block evacs run — the DVE executes its
            # stream in order, so the first TT must precede them there
            for ob in range(2):
                nc.vector.tensor_copy(
                    out=_ap(c2[:, 2 * o_sf + ob:2 * o_sf + ob + 1], [(2, P_sf)]),
                    in_=ps_t["c", ob][:, o_sf:o_sf + P_sf],
                )
                nc.scalar.copy(
                    out=_ap(yb[:, 2 * o_sf + ob:2 * o_sf + ob + 1], [(2, P_sf)]),
                    in_=ps_t["y", ob][:, o_sf:o_sf + P_sf],
                )
            pts = {sorder[0]: _pass1(sorder[0])}
            for ob in range(2):
                nc.vector.tensor_copy(
                    out=_ap(c2[:, ob:ob + 1], [(2, o_sf)]),
                    in_=ps_t["c", ob][:, 0:o_sf],
                )
                nc.scalar.copy(
                    out=_ap(yb[:, ob:ob + 1], [(2, o_sf)]),
                    in_=ps_t["y", ob][:, 0:o_sf],
                )
            for si in sorder[1:]:
                pts[si] = _pass1(si)

            # ---- pass 2: sigmoid -> halving tree -> reduce -> store
            for si in sorder:
                P, PI = P_list[si], PI_list[si]
                col = offs[si]
                pt = pts[si]
                eng = nc.vector
                # sigmoid stays fully contiguous in the pairs layout (a
                # strided ACT output measured ~4x slower)
                st = stp.tile([128, PI, P, 2], BF16, tag="sig", name="sig_t")
                nc.scalar.activation(out=st[:], in_=pt[:], func=AT.Sigmoid)
                # halving tree in pairs layout: the j offset is always
                # 2*j elements = 4j bytes, so every level keeps 2x packing
                hw = P // 2
                hv = hvp.tile([128, PI, hw, 2], BF16, tag="hv", name="hv_t")
                eng.tensor_tensor(
                    out=hv[:], in0=st[:, :, 0:hw, :], in1=st[:, :, hw:P, :],
                    op=OP.add,
                )
                last = hv
                w = hw
                if hw % 2 == 0:
                    hq = hw // 2
                    h2 = hqp.tile([128, PI, hq, 2], BF16, tag="hq", name="hq_t")
                    eng.tensor_tensor(
                        out=h2[:], in0=hv[:, :, 0:hq, :], in1=hv[:, :, hq:hw, :],
                        op=OP.add,
                    )
                    last, w = h2, hq
                # tensor_reduce is 1x (stride-insensitive): iterate the pairs
                # tile as (i, ob, j) via the input AP so X reduces over j,
                # and write red ob-major so the store DMA stays contiguous
                red = oup.tile([128, 2, PI], F32, tag="red", name="red_t")
                if si == sorder[-1]:
                    # last slot: reduce+store per ob so the final (tail-
                    # bounding) store is half-sized and issues earlier
                    for ob in range(2):
                        eng.reduce_sum(
                            out=_ap(red[:, ob:ob + 1], [(1, PI)]),
                            in_=_ap(last[:, 0:1, 0:1, ob:ob + 1], [(2 * w, PI), (2, w)]),
                            axis=mybir.AxisListType.X,
                        )
                        nc.sync.dma_start(
                            out=out[0:128, ob * NTOT + col:ob * NTOT + col + PI],
                            in_=_ap(red[:, ob:ob + 1], [(1, PI)]),
                        )
                else:
                    eng.reduce_sum(
                        out=_ap(red[:, 0:1], [(1, PI), (PI, 2)]),
                        in_=_ap(last[:, 0:1, 0:1, 0:1], [(2 * w, PI), (1, 2), (2, w)]),
                        axis=mybir.AxisListType.X,
                    )
                    nc.sync.dma_start(
                        out=_ap(out[0:128, col:col + PI], [(NTOT, 2), (1, PI)]),
                        in_=red[:],
                    )

    nc.finalize()
    return nc


def kernel(num_graphs, nv, z, mask, Wc, bc, Wy, by):
    global _last_results
    G = int(num_graphs)
    NV = int(nv)
    z = np.ascontiguousarray(np.asarray(z, dtype=np.float32))
    mask = np.asarray(mask, dtype=np.float32).reshape(G, NV)
    Wc = np.asarray(Wc, dtype=np.float32)
    bc = np.asarray(bc, dtype=np.float32)
    Wy = np.asarray(Wy, dtype=np.float32)
    by = np.asarray(by, dtype=np.float32)
    H = z.shape[-1]
    zg = z.reshape(G, NV, H)

    out_full = np.zeros((G * NV, H), dtype=np.float32)

    # ---- host: active-node compaction & slot assignment ----
    act_idx = [np.nonzero(mask[g] > 0.5)[0] for g in range(G)]
    n_act = np.array([len(a) for a in act_idx])
    for g in range(G):
        if n_act[g] == 0:  # reference: 0/0 -> NaN for the whole graph
            out_full[g * NV:(g + 1) * NV, :] = np.nan

    order = np.argsort(-n_act, kind="stable")  # graphs by count, descending
    n_slots = (G + N_CORES - 1) // N_CORES
    assign = [[None] * n_slots for _ in range(N_CORES)]
    P_list = []
    for s in range(n_slots):
        ranks = order[s * N_CORES:(s + 1) * N_CORES]
        for c, g in enumerate(ranks):
            assign[c][s] = int(g)
        mx = max((int(n_act[g]) for g in ranks), default=0)
        P_list.append(max(4, (mx + 3) // 4 * 4))  # j-extent: multiple of 4
    PI_list = [max(1, max((int(n_act[g]) for g in order[s * N_CORES:(s + 1) * N_CORES]), default=1)) for s in range(n_slots)]
    offs = np.cumsum([0] + P_list[:-1]).tolist()
    NTOT = sum(P_list)
    ONB = max(128, NTOT)

    # ---- host: per-core input staging ----
    import ml_dtypes

    def _wchunks(wt):  # [256, 256] -> [128, 512] chunks (kb,ob)-major for ob0 first
        w2 = np.empty((128, 512), dtype=ml_dtypes.bfloat16)
        for ob in range(2):
            for kb in range(2):
                w2[:, (2 * ob + kb) * 128:(2 * ob + kb + 1) * 128] = (
                    wt[kb * 128:(kb + 1) * 128, ob * 128:(ob + 1) * 128]
                )
        return np.ascontiguousarray(w2)

    wcT = _wchunks(Wc.T.astype(ml_dtypes.bfloat16))  # [h_in, o] chunks
    wyT = _wchunks(Wy.T.astype(ml_dtypes.bfloat16))
    bsum = (bc + by).astype(np.float32)

    in_maps = []
    for c in range(N_CORES):
        zT_act = np.zeros((H, NTOT), dtype=ml_dtypes.bfloat16)
        madd = np.full((1, NTOT), PAD_NEG, dtype=np.float32)
        for s in range(n_slots):
            g = assign[c][s]
            if g is None:
                continue
            n = int(n_act[g])
            if n == 0:
                continue
            o = int(offs[s])
            zT_act[:, o:o + n] = zg[g][act_idx[g]].T.astype(ml_dtypes.bfloat16)
            madd[0, o:o + n] = 0.0
        blob = np.empty((128, 2 * NTOT + 512), dtype=ml_dtypes.bfloat16)
        blob[:, :NTOT] = zT_act[:128]
        blob[:, NTOT:2 * NTOT] = zT_act[128:]
        blob[:, 2 * NTOT:2 * NTOT + 512] = wcT
        auxrow = np.zeros((2, NTOT + 256), dtype=ml_dtypes.bfloat16)
        auxrow[0, 0:NTOT] = madd[0].astype(ml_dtypes.bfloat16)
        auxrow[0, NTOT:NTOT + 256] = 1.0
        auxrow[1, 0:NTOT] = 1.0
        auxrow[1, NTOT:NTOT + 256] = bsum.astype(ml_dtypes.bfloat16)
        in_maps.append(
            {
                "blob": np.ascontiguousarray(blob),
                "wyT": wyT,
                "aux": np.ascontiguousarray(auxrow),
            }
        )

    # ---- build + run ----
    key = (tuple(P_list), tuple(PI_list), H)
    nc = _program_cache.get(key)
    if nc is None:
        nc = _build_program(P_list, PI_list, H)
        _program_cache[key] = nc
    res = run_bass_kernel_spmd(nc, in_maps, list(range(N_CORES)))
    _last_results = res

    # ---- host: scatter back (device output is [h1, (ob, col)]-major) ----
    for c in range(N_CORES):
        oc = np.asarray(res.results[c]["out"], dtype=np.float32).reshape(128, 2, NTOT)
        for s in range(n_slots):
            g = assign[c][s]
            if g is None:
                continue
            n = int(n_act[g])
            if n == 0:
                continue
            o = int(offs[s])
            blk = oc[:, :, o:o + n]  # [128, 2, n] (unscaled sums)
            out_full[g * NV + act_idx[g], :] = (
                blk.transpose(2, 1, 0).reshape(n, H)
                * (np.float32(1.0) / np.float32(n))
            )
    return out_full
